# revision 31
# baseline (speedup 1.0000x reference)
"""Bass/Trainium2 kernel for naive causal multi-head attention.

Problem: B=4, S=2048, E=1024, H=16, DH=64 (fp32 in/out).

Sharding (8 NeuronCores): core c handles batch b = c//2 and head group
g = c%2 (heads 8g..8g+7).  Each core computes its 8 heads' attention for
its batch plus the partial out-projection through its 512 columns of the
concat dim; the host sums the two partial outputs per batch.

Device-side dataflow (all matmuls bf16, PSUM fp32).  Heads are processed
in PAIRS (hA=2p, hB=2p+1): q/kT of the pair live stacked on partitions
0:64 / 64:128, and the two heads' score matmuls (contraction dim 64)
run CONCURRENTLY in the PE array as row-tiles (0,0)/(64,0) — 2x score
throughput vs one K=64 matmul at a time.

  q/kT pair = Wq_pair | Wk_pair @ xT  -> q_pair/kt_pair [128,S]   (PE)
  v'   = x @ Wv^T (+ ones column per head)                        (PE)
  sT   = ktile^T q  -> [128 keys, 512 q] x 2 heads row-tiled      (PE)
  p    = exp(sT/8)   (PSUM->SBUF, diag-masked)                    (ACT, DVE)
  oT   = v'^T p      -> [65, 512]  (row 64 = sums)                (PE, accum)
  rec  = 1/sums read straight from PSUM; bcast via gpsimd         (DVE/POOL)
  cT   = oT * rec                                                 (DVE)
  out  = concatT^T @ WoT  -> bf16, host sums the 2 partials       (PE)

The outer loop is qt-major (query tile 0..3, pairs inner) so the
out-projection for finished token blocks drips into the last query
tile's ACT-bound attention bubbles instead of running as a serial tail
(which previously sent the PE HAM clock-gate back to K=4/8).
"""

import numpy as np
import ml_dtypes

import concourse.bacc as bacc
import concourse.bass as bass
import concourse.mybir as mybir
from concourse.tile import TileContext
from concourse.bass_utils import run_bass_kernel_spmd


F32 = mybir.dt.float32
BF16 = mybir.dt.bfloat16
EXP = mybir.ActivationFunctionType.Exp

N_CORES = 8
BF = ml_dtypes.bfloat16


def build_nc(S=2048, E=1024, HPC=8, DH=64):
    """Build the per-core Bass program (identical on all cores)."""
    NQ = 512                      # query-tile width
    nst = S // 128                # key tiles
    nec = E // 128                # e chunks (contraction tiles)
    nqt = S // NQ                 # query tiles
    HD = HPC * DH                 # local concat width (512)
    ncc = HD // 128               # concat chunks (4)
    NP = HPC // 2                 # head pairs (4)
    assert NQ == 512 and S % 512 == 0

    nc = bacc.Bacc("TRN2", target_bir_lowering=False, debug=False,
                   num_devices=N_CORES)

    xtd = nc.dram_tensor("xt", [128, nqt, nec, 512], BF16,
                         kind="ExternalInput")
    wqkt = nc.dram_tensor("wqkt", [128, NP, nec, 256], BF16,
                          kind="ExternalInput")
    wvt = nc.dram_tensor("wvt", [128, nec, HD], BF16, kind="ExternalInput")
    wot = nc.dram_tensor("wot", [128, ncc, E], BF16, kind="ExternalInput")
    maskab = nc.dram_tensor("maskab", [128, 4, 1024], BF16,
                            kind="ExternalInput")
    out = nc.dram_tensor("out", [S, E], BF16, kind="ExternalOutput")

    with TileContext(nc) as tc:
        with (
            tc.tile_pool(name="persist", bufs=1) as persist,
            tc.tile_pool(name="ptp", bufs=8) as ptp,
            tc.tile_pool(name="prawp", bufs=6) as prawp,
            tc.tile_pool(name="recp", bufs=6) as recp,
            tc.tile_pool(name="bcp", bufs=4) as bcp,
            tc.tile_pool(name="outp", bufs=3) as outp,
            tc.tile_pool(name="ps_c1", bufs=2, space="PSUM") as ps_c1,
            tc.tile_pool(name="ps_big", bufs=2, space="PSUM") as ps_big,
            tc.tile_pool(name="ps_o", bufs=2, space="PSUM") as ps_o,
        ):
            # ---- persistent SBUF tensors ----
            xT = persist.tile([128, nqt, nec, 512], BF16)
            wqk = persist.tile([128, NP, nec, 256], BF16)
            wv = persist.tile([128, nec, HD], BF16)
            wo = persist.tile([128, ncc, E], BF16)
            vS = persist.tile([128, nst, HPC * (DH + 1)], BF16)
            cT = persist.tile([128, ncc, S], BF16)
            mk = persist.tile([128, 4, 1024], BF16)
            # q/kT for all 4 pairs stay resident (qt-major loop)
            qp = persist.tile([128, NP, S], BF16)
            ktp = persist.tile([128, NP, S], BF16)

            # ---- phase A: scrub + warmup + DMAs ----
            # Scrub the exp-source PSUM banks: narrowed diagonal score
            # blocks leave columns unwritten; exp() of stale garbage can
            # reach inf (then 0*inf=NaN in the mask).  Zero once.
            for i in range(2):
                scrub = ps_big.tile([128, 1024], F32, tag="sbig",
                                    name=f"scrub{i}")
                nc.vector.memset(scrub, 0.0)
            # PE warmup: dummy matmul burst inside the DMA-ramp window
            # trips the HAM clock gate so the real stream starts warm.
            warm = persist.tile([128, 512], BF16)
            nc.vector.memset(warm, 1.0)
            pwarm = ps_c1.tile([128, 512], F32, tag="c1")
            for i in range(16):
                nc.tensor.matmul(pwarm, lhsT=warm[:, 0:128], rhs=warm,
                                 start=(i == 0), stop=(i == 15))
            nc.vector.memset(vS, 1.0)
            # finer first chunks so the first projection chain starts early
            nc.sync.dma_start(out=wqk[:, 0, 0:2], in_=wqkt[:, 0, 0:2])
            nc.sync.dma_start(out=xT[:, 0, 0:2], in_=xtd[:, 0, 0:2])
            nc.sync.dma_start(out=wqk[:, 0, 2:nec], in_=wqkt[:, 0, 2:nec])
            nc.sync.dma_start(out=xT[:, 0, 2:nec], in_=xtd[:, 0, 2:nec])
            nc.sync.dma_start(out=wqk[:, 1:NP], in_=wqkt[:, 1:NP])
            nc.sync.dma_start(out=wv[:, 0:nec // 2], in_=wvt[:, 0:nec // 2])
            nc.sync.dma_start(out=wv[:, nec // 2:], in_=wvt[:, nec // 2:])
            nc.sync.dma_start(out=mk, in_=maskab[:, :, :])
            nc.sync.dma_start(out=xT[:, 1], in_=xtd[:, 1])
            for j in range(2, nqt):
                nc.sync.dma_start(out=xT[:, j], in_=xtd[:, j])
            nc.sync.dma_start(out=wo, in_=wot[:, :, :])

            # ---- filler emitters (dripped into attention PE bubbles) ----
            def gen_proj(p, sc):
                """q then k projection chain of pair p, x-chunk sc."""
                for half in range(2):  # 0 = q pair, 1 = k pair
                    ps = ps_c1.tile([128, 512], F32, tag="c1")
                    for ec in range(nec):
                        nc.tensor.matmul(
                            ps, lhsT=wqk[:, p, ec,
                                         128 * half:128 * half + 128],
                            rhs=xT[:, sc, ec, :],
                            start=(ec == 0), stop=(ec == nec - 1))
                        yield
                    dst = qp if half == 0 else ktp
                    nc.vector.tensor_copy(
                        out=dst[:, p, sc * 512:(sc + 1) * 512], in_=ps)

            def gen_vproj(st):
                """v projection for one 128-token tile st."""
                pv = ps_c1.tile([128, HD], F32, tag="c1")
                for ec in range(nec):
                    nc.tensor.matmul(
                        pv,
                        lhsT=xT[:, st // 4, ec,
                                (st % 4) * 128:(st % 4 + 1) * 128],
                        rhs=wv[:, ec], start=(ec == 0),
                        stop=(ec == nec - 1))
                    yield
                nc.vector.tensor_copy(
                    out=vS[:, st].rearrange("p (h m) -> p h m",
                                            m=DH + 1)[:, :, 0:DH],
                    in_=pv.rearrange("p (h m) -> p h m", m=DH))

            def gen_outproj(st):
                """partial out-projection for token tile st (bf16 out).
                n2 inner so the second matmul reuses the cT stationary
                (ldweights=False) -- halves the exposed weight loads."""
                osb = outp.tile([128, E], BF16, tag="osb")
                pd = [ps_c1.tile([128, 512], F32, tag="c1", name=f"pd{st}_{n}")
                      for n in range(2)]
                for c in range(ncc):
                    for n2 in range(2):
                        mm = nc.tensor.matmul(
                            pd[n2],
                            lhsT=cT[:, c, st * 128:(st + 1) * 128],
                            rhs=wo[:, c, n2 * 512:(n2 + 1) * 512],
                            start=(c == 0), stop=(c == ncc - 1),
                            skip_group_check=True)
                        if n2 == 1:  # same stationary as the n2=0 matmul
                            mm.ins.ldweights = False
                        yield
                for n2 in range(2):
                    nc.vector.tensor_copy(
                        out=osb[:, n2 * 512:(n2 + 1) * 512], in_=pd[n2])
                nc.sync.dma_start(out=out[st * 128:(st + 1) * 128, :],
                                  in_=osb)

            class FillQueue:
                """Filler MMs dripped into attention PE bubbles, spread
                evenly over the slots so the PE never idles long enough
                to trip the HAM MID re-throttle."""

                def __init__(self):
                    self.gens = []
                    self.remaining = 0
                    self.slots = 0

                def add(self, g, n):
                    self.gens.append(g)
                    self.remaining += n

                def set_slots(self, n):
                    self.slots = n

                def fill(self, n=1):
                    for _ in range(n):
                        while self.gens:
                            try:
                                next(self.gens[0])
                                self.remaining -= 1
                                break
                            except StopIteration:
                                self.gens.pop(0)

                def fill_auto(self):
                    if self.slots > 0:
                        q = -(-self.remaining // self.slots)  # ceil
                        self.slots -= 1
                        self.fill(min(q, 4))

                def drain(self):
                    while self.gens:
                        self.fill()

            def attention(p, qt, fq):
                """one query tile of head pair p (row-tiled scores)."""
                hA, hB = 2 * p, 2 * p + 1
                ngrp = 4 * qt + 4
                po_a = ps_o.tile([DH + 1, 512], F32, tag="o")
                po_b = ps_o.tile([DH + 1, 512], F32, tag="o")
                pts = {}

                def emit_scores(g):
                    d = g - 4 * qt
                    n0 = 128 * d if d > 0 else 0
                    ps2 = ps_big.tile([128, 1024], F32, tag="sbig")
                    nc.tensor.matmul(
                        ps2[:, n0:512],
                        lhsT=ktp[0:64, p, g * 128:(g + 1) * 128],
                        rhs=qp[0:64, p, qt * 512 + n0:(qt + 1) * 512],
                        start=True, stop=True)
                    nc.tensor.matmul(
                        ps2[:, 512 + n0:1024],
                        lhsT=ktp[64:128, p, g * 128:(g + 1) * 128],
                        rhs=qp[64:128, p, qt * 512 + n0:(qt + 1) * 512],
                        start=True, stop=True)
                    pt = ptp.tile([128, 1024], BF16, tag="pt")
                    if n0 == 0:
                        nc.scalar.activation(out=pt, in_=ps2, func=EXP,
                                             scale=0.125)
                        if d == 0:  # diagonal tile needs causal mask
                            nc.vector.tensor_mul(pt, pt, mk[:, d, :])
                    else:
                        # touch ONLY the columns the matmuls wrote: AP with
                        # two [n0:512] regions 512 apart (skips the masked
                        # corners -> less ACT/DVE work, no stale reads)
                        def reg2(ap):
                            return ap.rearrange(
                                "p (a c) -> p a c", a=2)[:, :, n0:512]
                        nc.scalar.activation(out=reg2(pt), in_=reg2(ps2),
                                             func=EXP, scale=0.125)
                        nc.vector.tensor_mul(reg2(pt), reg2(pt),
                                             reg2(mk[:, d, :]))
                    pts[g] = pt

                def emit_av(g):
                    pt = pts.pop(g)
                    d = g - 4 * qt
                    n0 = 128 * d if d > 0 else 0
                    nc.tensor.matmul(
                        po_a[:, n0:512],
                        lhsT=vS[:, g, hA * (DH + 1):(hA + 1) * (DH + 1)],
                        rhs=pt[:, n0:512],
                        start=(g == 0), stop=(g == ngrp - 1),
                        skip_group_check=True)
                    nc.tensor.matmul(
                        po_b[:, n0:512],
                        lhsT=vS[:, g, hB * (DH + 1):(hB + 1) * (DH + 1)],
                        rhs=pt[:, 512 + n0:1024],
                        start=(g == 0), stop=(g == ngrp - 1),
                        skip_group_check=True)

                # 3-deep software pipeline: av lags scores by three blocks
                # (scores emission is still gated 2-deep by the PSUM pool;
                # the extra lag buys the exp->mask chain another block of
                # latency slack before the AV matmuls need the result).
                lag = min(3, ngrp - 1)
                for g in range(lag):
                    emit_scores(g)
                for g in range(lag, ngrp):
                    emit_scores(g)
                    emit_av(g - lag)
                    fq.fill_auto()
                for g in range(ngrp - lag, ngrp):
                    emit_av(g)
                    if g < ngrp - 1:
                        fq.fill_auto()

                def normalize(h, po):
                    # sums row is copied to partition 0 first: the custom-DVE
                    # fast reciprocal misreads inputs at base partition 64 on
                    # hardware (fine in CoreSim).  The mul reads po straight
                    # from PSUM; the 2-block AV lag gives the pool enough
                    # slack to absorb the bank hold.
                    sums = recp.tile([1, 512], F32, tag="sums")
                    nc.vector.tensor_copy(out=sums, in_=po[DH:DH + 1, :])
                    rec = recp.tile([1, 512], F32, tag="rec")
                    nc.vector.reciprocal_approx_fast(out=rec, in_=sums)
                    bc = bcp.tile([64, 512], F32, tag="bc")
                    nc.gpsimd.partition_broadcast(bc, rec)
                    nc.vector.tensor_mul(
                        cT[64 * (h % 2):64 * (h % 2) + 64, h // 2,
                           qt * 512:(qt + 1) * 512],
                        po[0:DH, :], bc)

                normalize(hA, po_a)
                normalize(hB, po_b)

            # ---- upfront: projections for x-chunk 0 + v for tiles 0-3 ----
            for p in range(NP):
                for _ in gen_proj(p, 0):
                    pass
            for st in range(4):
                for _ in gen_vproj(st):
                    pass

            # ---- qt-major attention with dripped fillers ----
            # Filler budget per qt (slots = 4 pairs x (ngrp-1)):
            #  qt0 (24 slots): proj sc1 (64) + vproj st4-7 (32) -> surplus
            #      drains inline after the pair loop (PE-dense, ACT idles)
            #  qt1 (56): proj sc2 (64) + vproj st8-11 (32)
            #  qt2 (88): proj sc3 pairs 0,1 (32) + vproj st12-15 (32)
            #  qt3 (120): proj sc3 pairs 2,3 (32, front -> done before
            #      pair 2 attends) + outproj st0-11 (96)
            for qt in range(nqt):
                fq = FillQueue()
                if qt == 0:
                    for p in range(NP):
                        fq.add(gen_proj(p, 1), 2 * nec)
                    for st in range(4, 8):
                        fq.add(gen_vproj(st), nec)
                elif qt == 1:
                    for p in range(NP):
                        fq.add(gen_proj(p, 2), 2 * nec)
                    for st in range(8, 12):
                        fq.add(gen_vproj(st), nec)
                elif qt == 2:
                    for p in range(2):
                        fq.add(gen_proj(p, 3), 2 * nec)
                    for st in range(12, 16):
                        fq.add(gen_vproj(st), nec)
                else:  # qt == 3: out-projection of finished token tiles
                    for p in range(2, NP):
                        fq.add(gen_proj(p, 3), 2 * nec)
                    for st in range(0, 11):
                        fq.add(gen_outproj(st), 2 * ncc)
                fq.set_slots(NP * (4 * qt + 3))
                for p in range(NP):
                    attention(p, qt, fq)
                fq.drain()

            # ---- tail: st11 (ready tokens) covers the last pair's
            # normalize latency, then the final qt3 token tiles ----
            for st in range(11, nst):
                for _ in gen_outproj(st):
                    pass

    nc.finalize()
    return nc


def _make_masks():
    """[128, 4, 1024] bf16 diag masks, duplicated for the 2-head block."""
    j = np.arange(128)[:, None]
    i = np.arange(512)[None, :]
    out = np.empty((128, 4, 1024), dtype=np.float32)
    for d in range(4):
        m = (j <= i - 128 * d).astype(np.float32)
        out[:, d, 0:512] = m
        out[:, d, 512:1024] = m
    return np.ascontiguousarray(out).astype(BF)


def _host_prep(x, Wq, Wk, Wv, Wo, HPC=8, DH=64):
    """Build the 8 per-core input maps (bf16, pre-transposed)."""
    B, S, E = x.shape
    nec = E // 128
    HD = HPC * DH
    NP = HPC // 2
    masks = _make_masks()
    xts = []
    for b in range(B):
        xt = x[b].T.reshape(nec, 128, S // 512, 512)
        xt = xt.transpose(1, 2, 0, 3)   # [128, nqt, nec, 512]
        xts.append(np.ascontiguousarray(xt).astype(BF))
    in_maps = []
    for c in range(N_CORES):
        b, g = c // 2, c % 2
        h0 = HPC * g
        # per pair p: stationaries [q pair | k pair] along the out dim.
        # wqkt[pp, p, ec, cc]: cc 0:128 -> q dims of (hA,hB), 128:256 -> k.
        wqk = np.empty((128, NP, nec, 256), dtype=np.float32)
        for p in range(NP):
            hA, hB = h0 + 2 * p, h0 + 2 * p + 1
            qblk = np.concatenate([Wq[hA], Wq[hB]], axis=0)  # [128, E]
            kblk = np.concatenate([Wk[hA], Wk[hB]], axis=0)
            # [E, 128] -> [nec, 128pp, 128cc] -> [128pp, nec, 128cc]
            wqk[:, p, :, 0:128] = qblk.T.reshape(nec, 128, 128)\
                .transpose(1, 0, 2)
            wqk[:, p, :, 128:256] = kblk.T.reshape(nec, 128, 128)\
                .transpose(1, 0, 2)
        wqkt = np.ascontiguousarray(wqk).astype(BF)
        hs = slice(h0, h0 + HPC)
        # Wv slice -> [128, ec, HD]
        wvt = Wv[hs].transpose(2, 0, 1).reshape(nec, 128, HD)
        wvt = np.ascontiguousarray(wvt.transpose(1, 0, 2)).astype(BF)
        # Wo columns slice, transposed -> [128, ncc, E]
        wot = np.ascontiguousarray(Wo[:, HD * g:HD * (g + 1)].T)  # [HD, E]
        wot = np.ascontiguousarray(
            wot.reshape(HD // 128, 128, E).transpose(1, 0, 2)).astype(BF)
        in_maps.append({
            "xt": xts[b],
            "wqkt": wqkt, "wvt": wvt, "wot": wot, "maskab": masks,
        })
    return in_maps


_NC_CACHE = {}


def kernel(x, Wq, Wk, Wv, Wo):
    x = np.asarray(x, dtype=np.float32)
    Wq = np.asarray(Wq, dtype=np.float32)
    Wk = np.asarray(Wk, dtype=np.float32)
    Wv = np.asarray(Wv, dtype=np.float32)
    Wo = np.asarray(Wo, dtype=np.float32)
    B, S, E = x.shape
    H, DH, _ = Wq.shape
    HPC = H // 2

    key = (S, E, HPC, DH)
    if key not in _NC_CACHE:
        _NC_CACHE[key] = build_nc(S=S, E=E, HPC=HPC, DH=DH)
    nc = _NC_CACHE[key]

    in_maps = _host_prep(x, Wq, Wk, Wv, Wo, HPC=HPC, DH=DH)
    res = run_bass_kernel_spmd(nc, in_maps, core_ids=list(range(N_CORES)))
    kernel.last_results = res

    out = np.empty((B, S, E), dtype=np.float32)
    for b in range(B):
        out[b] = (res.results[2 * b]["out"].astype(np.float32)
                  + res.results[2 * b + 1]["out"].astype(np.float32))
    return out


# revision 32
# speedup vs baseline: 1.0073x; 1.0073x over previous
"""Bass/Trainium2 kernel for naive causal multi-head attention.

Problem: B=4, S=2048, E=1024, H=16, DH=64 (fp32 in/out).

Sharding (8 NeuronCores): core c handles batch b = c//2 and head group
g = c%2 (heads 8g..8g+7).  Each core computes its 8 heads' attention for
its batch plus the partial out-projection through its 512 columns of the
concat dim; the host sums the two partial outputs per batch.

Device-side dataflow (all matmuls bf16, PSUM fp32).  Heads are processed
in PAIRS (hA=2p, hB=2p+1): q/kT of the pair live stacked on partitions
0:64 / 64:128, and the two heads' score matmuls (contraction dim 64)
run CONCURRENTLY in the PE array as row-tiles (0,0)/(64,0) — 2x score
throughput vs one K=64 matmul at a time.

  q/kT pair = Wq_pair | Wk_pair @ xT  -> q_pair/kt_pair [128,S]   (PE)
  v'   = x @ Wv^T (+ ones column per head)                        (PE)
  sT   = ktile^T q  -> [128 keys, 512 q] x 2 heads row-tiled      (PE)
  p    = exp(sT/8)   (PSUM->SBUF, diag-masked)                    (ACT, DVE)
  oT   = v'^T p      -> [65, 512]  (row 64 = sums)                (PE, accum)
  rec  = 1/sums read straight from PSUM; bcast via gpsimd         (DVE/POOL)
  cT   = oT * rec                                                 (DVE)
  out  = concatT^T @ WoT  -> bf16, host sums the 2 partials       (PE)

The outer loop is qt-major (query tile 0..3, pairs inner) so the
out-projection for finished token blocks drips into the last query
tile's ACT-bound attention bubbles instead of running as a serial tail
(which previously sent the PE HAM clock-gate back to K=4/8).
"""

import numpy as np
import ml_dtypes

import concourse.bacc as bacc
import concourse.bass as bass
import concourse.mybir as mybir
from concourse.tile import TileContext
from concourse.bass_utils import run_bass_kernel_spmd


F32 = mybir.dt.float32
BF16 = mybir.dt.bfloat16
EXP = mybir.ActivationFunctionType.Exp

N_CORES = 8
BF = ml_dtypes.bfloat16


def build_nc(S=2048, E=1024, HPC=8, DH=64):
    """Build the per-core Bass program (identical on all cores)."""
    NQ = 512                      # query-tile width
    nst = S // 128                # key tiles
    nec = E // 128                # e chunks (contraction tiles)
    nqt = S // NQ                 # query tiles
    HD = HPC * DH                 # local concat width (512)
    ncc = HD // 128               # concat chunks (4)
    NP = HPC // 2                 # head pairs (4)
    assert NQ == 512 and S % 512 == 0

    nc = bacc.Bacc("TRN2", target_bir_lowering=False, debug=False,
                   num_devices=N_CORES)

    xtd = nc.dram_tensor("xt", [128, nqt, nec, 512], BF16,
                         kind="ExternalInput")
    wqkt = nc.dram_tensor("wqkt", [128, NP, nec, 256], BF16,
                          kind="ExternalInput")
    wvt = nc.dram_tensor("wvt", [128, nec, HD], BF16, kind="ExternalInput")
    wot = nc.dram_tensor("wot", [128, ncc, E], BF16, kind="ExternalInput")
    maskab = nc.dram_tensor("maskab", [128, 4, 1024], BF16,
                            kind="ExternalInput")
    out = nc.dram_tensor("out", [S, E], BF16, kind="ExternalOutput")

    with TileContext(nc) as tc:
        with (
            tc.tile_pool(name="persist", bufs=1) as persist,
            tc.tile_pool(name="ptp", bufs=8) as ptp,
            tc.tile_pool(name="prawp", bufs=6) as prawp,
            tc.tile_pool(name="recp", bufs=6) as recp,
            tc.tile_pool(name="bcp", bufs=4) as bcp,
            tc.tile_pool(name="outp", bufs=3) as outp,
            tc.tile_pool(name="ps_c1", bufs=2, space="PSUM") as ps_c1,
            tc.tile_pool(name="ps_big", bufs=2, space="PSUM") as ps_big,
            tc.tile_pool(name="ps_o", bufs=2, space="PSUM") as ps_o,
        ):
            # ---- persistent SBUF tensors ----
            xT = persist.tile([128, nqt, nec, 512], BF16)
            wqk = persist.tile([128, NP, nec, 256], BF16)
            wv = persist.tile([128, nec, HD], BF16)
            wo = persist.tile([128, ncc, E], BF16)
            vS = persist.tile([128, nst, HPC * (DH + 1)], BF16)
            cT = persist.tile([128, ncc, S], BF16)
            mk = persist.tile([128, 4, 1024], BF16)
            # q/kT for all 4 pairs stay resident (qt-major loop)
            qp = persist.tile([128, NP, S], BF16)
            ktp = persist.tile([128, NP, S], BF16)

            # ---- phase A: scrub + warmup + DMAs ----
            # Scrub the exp-source PSUM banks: narrowed diagonal score
            # blocks leave columns unwritten; exp() of stale garbage can
            # reach inf (then 0*inf=NaN in the mask).  Zero once.
            for i in range(2):
                scrub = ps_big.tile([128, 1024], F32, tag="sbig",
                                    name=f"scrub{i}")
                nc.vector.memset(scrub, 0.0)
            # PE warmup: dummy matmul burst inside the DMA-ramp window
            # trips the HAM clock gate so the real stream starts warm.
            warm = persist.tile([128, 512], BF16)
            nc.vector.memset(warm, 1.0)
            pwarm = ps_c1.tile([128, 512], F32, tag="c1")
            for i in range(16):
                nc.tensor.matmul(pwarm, lhsT=warm[:, 0:128], rhs=warm,
                                 start=(i == 0), stop=(i == 15))
            nc.vector.memset(vS, 1.0)
            # finer first chunks so the first projection chain starts early
            nc.sync.dma_start(out=wqk[:, 0, 0:2], in_=wqkt[:, 0, 0:2])
            nc.sync.dma_start(out=xT[:, 0, 0:2], in_=xtd[:, 0, 0:2])
            nc.sync.dma_start(out=wqk[:, 0, 2:nec], in_=wqkt[:, 0, 2:nec])
            nc.sync.dma_start(out=xT[:, 0, 2:nec], in_=xtd[:, 0, 2:nec])
            nc.sync.dma_start(out=wqk[:, 1:NP], in_=wqkt[:, 1:NP])
            nc.sync.dma_start(out=wv[:, 0:nec // 2], in_=wvt[:, 0:nec // 2])
            nc.sync.dma_start(out=wv[:, nec // 2:], in_=wvt[:, nec // 2:])
            nc.sync.dma_start(out=mk, in_=maskab[:, :, :])
            nc.sync.dma_start(out=xT[:, 1], in_=xtd[:, 1])
            for j in range(2, nqt):
                nc.sync.dma_start(out=xT[:, j], in_=xtd[:, j])
            nc.sync.dma_start(out=wo, in_=wot[:, :, :])

            # ---- filler emitters (dripped into attention PE bubbles) ----
            def gen_proj(p, sc):
                """q then k projection chain of pair p, x-chunk sc."""
                for half in range(2):  # 0 = q pair, 1 = k pair
                    ps = ps_c1.tile([128, 512], F32, tag="c1")
                    for ec in range(nec):
                        nc.tensor.matmul(
                            ps, lhsT=wqk[:, p, ec,
                                         128 * half:128 * half + 128],
                            rhs=xT[:, sc, ec, :],
                            start=(ec == 0), stop=(ec == nec - 1))
                        yield
                    dst = qp if half == 0 else ktp
                    nc.vector.tensor_copy(
                        out=dst[:, p, sc * 512:(sc + 1) * 512], in_=ps)

            def gen_vproj(st):
                """v projection for one 128-token tile st."""
                pv = ps_c1.tile([128, HD], F32, tag="c1")
                for ec in range(nec):
                    nc.tensor.matmul(
                        pv,
                        lhsT=xT[:, st // 4, ec,
                                (st % 4) * 128:(st % 4 + 1) * 128],
                        rhs=wv[:, ec], start=(ec == 0),
                        stop=(ec == nec - 1))
                    yield
                nc.vector.tensor_copy(
                    out=vS[:, st].rearrange("p (h m) -> p h m",
                                            m=DH + 1)[:, :, 0:DH],
                    in_=pv.rearrange("p (h m) -> p h m", m=DH))

            def gen_outproj(st):
                """partial out-projection for token tile st (bf16 out)."""
                osb = outp.tile([128, E], BF16, tag="osb")
                for n2 in range(2):
                    pd = ps_c1.tile([128, 512], F32, tag="c1")
                    for c in range(ncc):
                        nc.tensor.matmul(
                            pd,
                            lhsT=cT[:, c, st * 128:(st + 1) * 128],
                            rhs=wo[:, c, n2 * 512:(n2 + 1) * 512],
                            start=(c == 0), stop=(c == ncc - 1),
                            skip_group_check=True)
                        yield
                    nc.vector.tensor_copy(
                        out=osb[:, n2 * 512:(n2 + 1) * 512], in_=pd)
                nc.sync.dma_start(out=out[st * 128:(st + 1) * 128, :],
                                  in_=osb)

            class FillQueue:
                """Filler MMs dripped into attention PE bubbles, spread
                evenly over the slots so the PE never idles long enough
                to trip the HAM MID re-throttle."""

                def __init__(self):
                    self.gens = []
                    self.remaining = 0
                    self.slots = 0

                def add(self, g, n):
                    self.gens.append(g)
                    self.remaining += n

                def set_slots(self, n):
                    self.slots = n

                def fill(self, n=1):
                    for _ in range(n):
                        while self.gens:
                            try:
                                next(self.gens[0])
                                self.remaining -= 1
                                break
                            except StopIteration:
                                self.gens.pop(0)

                def fill_auto(self):
                    if self.slots > 0:
                        q = -(-self.remaining // self.slots)  # ceil
                        self.slots -= 1
                        self.fill(min(q, 4))

                def drain(self):
                    while self.gens:
                        self.fill()

            def attention(p, qt, fq):
                """one query tile of head pair p (row-tiled scores)."""
                hA, hB = 2 * p, 2 * p + 1
                ngrp = 4 * qt + 4
                po_a = ps_o.tile([DH + 1, 512], F32, tag="o")
                po_b = ps_o.tile([DH + 1, 512], F32, tag="o")
                pts = {}

                def emit_scores(g):
                    d = g - 4 * qt
                    n0 = 128 * d if d > 0 else 0
                    ps2 = ps_big.tile([128, 1024], F32, tag="sbig")
                    nc.tensor.matmul(
                        ps2[:, n0:512],
                        lhsT=ktp[0:64, p, g * 128:(g + 1) * 128],
                        rhs=qp[0:64, p, qt * 512 + n0:(qt + 1) * 512],
                        start=True, stop=True)
                    nc.tensor.matmul(
                        ps2[:, 512 + n0:1024],
                        lhsT=ktp[64:128, p, g * 128:(g + 1) * 128],
                        rhs=qp[64:128, p, qt * 512 + n0:(qt + 1) * 512],
                        start=True, stop=True)
                    pt = ptp.tile([128, 1024], BF16, tag="pt")
                    if n0 == 0:
                        nc.scalar.activation(out=pt, in_=ps2, func=EXP,
                                             scale=0.125)
                        if d == 0:  # diagonal tile needs causal mask
                            nc.vector.tensor_mul(pt, pt, mk[:, d, :])
                    else:
                        # touch ONLY the columns the matmuls wrote: AP with
                        # two [n0:512] regions 512 apart (skips the masked
                        # corners -> less ACT/DVE work, no stale reads)
                        def reg2(ap):
                            return ap.rearrange(
                                "p (a c) -> p a c", a=2)[:, :, n0:512]
                        nc.scalar.activation(out=reg2(pt), in_=reg2(ps2),
                                             func=EXP, scale=0.125)
                        nc.vector.tensor_mul(reg2(pt), reg2(pt),
                                             reg2(mk[:, d, :]))
                    pts[g] = pt

                def emit_av(g):
                    pt = pts.pop(g)
                    d = g - 4 * qt
                    n0 = 128 * d if d > 0 else 0
                    nc.tensor.matmul(
                        po_a[:, n0:512],
                        lhsT=vS[:, g, hA * (DH + 1):(hA + 1) * (DH + 1)],
                        rhs=pt[:, n0:512],
                        start=(g == 0), stop=(g == ngrp - 1),
                        skip_group_check=True)
                    nc.tensor.matmul(
                        po_b[:, n0:512],
                        lhsT=vS[:, g, hB * (DH + 1):(hB + 1) * (DH + 1)],
                        rhs=pt[:, 512 + n0:1024],
                        start=(g == 0), stop=(g == ngrp - 1),
                        skip_group_check=True)

                # 3-deep software pipeline: av lags scores by three blocks
                # (scores emission is still gated 2-deep by the PSUM pool;
                # the extra lag buys the exp->mask chain another block of
                # latency slack before the AV matmuls need the result).
                lag = min(3, ngrp - 1)
                for g in range(lag):
                    emit_scores(g)
                for g in range(lag, ngrp):
                    emit_scores(g)
                    emit_av(g - lag)
                    fq.fill_auto()
                for g in range(ngrp - lag, ngrp):
                    emit_av(g)
                    if g < ngrp - 1:
                        fq.fill_auto()

                def normalize(h, po):
                    # sums row is copied to partition 0 first: the custom-DVE
                    # fast reciprocal misreads inputs at base partition 64 on
                    # hardware (fine in CoreSim).  The mul reads po straight
                    # from PSUM; the 2-block AV lag gives the pool enough
                    # slack to absorb the bank hold.
                    sums = recp.tile([1, 512], F32, tag="sums")
                    nc.vector.tensor_copy(out=sums, in_=po[DH:DH + 1, :])
                    rec = recp.tile([1, 512], F32, tag="rec")
                    nc.vector.reciprocal_approx_fast(out=rec, in_=sums)
                    bc = bcp.tile([64, 512], F32, tag="bc")
                    nc.gpsimd.partition_broadcast(bc, rec)
                    nc.vector.tensor_mul(
                        cT[64 * (h % 2):64 * (h % 2) + 64, h // 2,
                           qt * 512:(qt + 1) * 512],
                        po[0:DH, :], bc)

                normalize(hA, po_a)
                normalize(hB, po_b)

            # ---- upfront: projections for x-chunk 0 + v for tiles 0-3 ----
            for p in range(NP):
                for _ in gen_proj(p, 0):
                    pass
            for st in range(4):
                for _ in gen_vproj(st):
                    pass

            # ---- qt-major attention with dripped fillers ----
            # Filler budget per qt (slots = 4 pairs x (ngrp-1)):
            #  qt0 (24 slots): proj sc1 (64) + vproj st4-7 (32) -> surplus
            #      drains inline after the pair loop (PE-dense, ACT idles)
            #  qt1 (56): proj sc2 (64) + vproj st8-11 (32)
            #  qt2 (88): proj sc3 pairs 0,1 (32) + vproj st12-15 (32)
            #  qt3 (120): proj sc3 pairs 2,3 (32, front -> done before
            #      pair 2 attends) + outproj st0-11 (96)
            for qt in range(nqt):
                fq = FillQueue()
                if qt == 0:
                    for p in range(NP):
                        fq.add(gen_proj(p, 1), 2 * nec)
                    for st in range(4, 8):
                        fq.add(gen_vproj(st), nec)
                elif qt == 1:
                    for p in range(NP):
                        fq.add(gen_proj(p, 2), 2 * nec)
                    for st in range(8, 12):
                        fq.add(gen_vproj(st), nec)
                elif qt == 2:
                    for p in range(2):
                        fq.add(gen_proj(p, 3), 2 * nec)
                    for st in range(12, 16):
                        fq.add(gen_vproj(st), nec)
                else:  # qt == 3: out-projection of finished token tiles
                    for p in range(2, NP):
                        fq.add(gen_proj(p, 3), 2 * nec)
                    for st in range(0, 11):
                        fq.add(gen_outproj(st), 2 * ncc)
                fq.set_slots(NP * (4 * qt + 3))
                for p in range(NP):
                    attention(p, qt, fq)
                fq.drain()

            # ---- tail: st11 (ready tokens) covers the last pair's
            # normalize latency, then the final qt3 token tiles ----
            for st in range(11, nst):
                for _ in gen_outproj(st):
                    pass

    nc.finalize()
    return nc


def _make_masks():
    """[128, 4, 1024] bf16 diag masks, duplicated for the 2-head block."""
    j = np.arange(128)[:, None]
    i = np.arange(512)[None, :]
    out = np.empty((128, 4, 1024), dtype=np.float32)
    for d in range(4):
        m = (j <= i - 128 * d).astype(np.float32)
        out[:, d, 0:512] = m
        out[:, d, 512:1024] = m
    return np.ascontiguousarray(out).astype(BF)


def _host_prep(x, Wq, Wk, Wv, Wo, HPC=8, DH=64):
    """Build the 8 per-core input maps (bf16, pre-transposed)."""
    B, S, E = x.shape
    nec = E // 128
    HD = HPC * DH
    NP = HPC // 2
    masks = _make_masks()
    xts = []
    for b in range(B):
        xt = x[b].T.reshape(nec, 128, S // 512, 512)
        xt = xt.transpose(1, 2, 0, 3)   # [128, nqt, nec, 512]
        xts.append(np.ascontiguousarray(xt).astype(BF))
    in_maps = []
    for c in range(N_CORES):
        b, g = c // 2, c % 2
        h0 = HPC * g
        # per pair p: stationaries [q pair | k pair] along the out dim.
        # wqkt[pp, p, ec, cc]: cc 0:128 -> q dims of (hA,hB), 128:256 -> k.
        wqk = np.empty((128, NP, nec, 256), dtype=np.float32)
        for p in range(NP):
            hA, hB = h0 + 2 * p, h0 + 2 * p + 1
            qblk = np.concatenate([Wq[hA], Wq[hB]], axis=0)  # [128, E]
            kblk = np.concatenate([Wk[hA], Wk[hB]], axis=0)
            # [E, 128] -> [nec, 128pp, 128cc] -> [128pp, nec, 128cc]
            wqk[:, p, :, 0:128] = qblk.T.reshape(nec, 128, 128)\
                .transpose(1, 0, 2)
            wqk[:, p, :, 128:256] = kblk.T.reshape(nec, 128, 128)\
                .transpose(1, 0, 2)
        wqkt = np.ascontiguousarray(wqk).astype(BF)
        hs = slice(h0, h0 + HPC)
        # Wv slice -> [128, ec, HD]
        wvt = Wv[hs].transpose(2, 0, 1).reshape(nec, 128, HD)
        wvt = np.ascontiguousarray(wvt.transpose(1, 0, 2)).astype(BF)
        # Wo columns slice, transposed -> [128, ncc, E]
        wot = np.ascontiguousarray(Wo[:, HD * g:HD * (g + 1)].T)  # [HD, E]
        wot = np.ascontiguousarray(
            wot.reshape(HD // 128, 128, E).transpose(1, 0, 2)).astype(BF)
        in_maps.append({
            "xt": xts[b],
            "wqkt": wqkt, "wvt": wvt, "wot": wot, "maskab": masks,
        })
    return in_maps


_NC_CACHE = {}


def kernel(x, Wq, Wk, Wv, Wo):
    x = np.asarray(x, dtype=np.float32)
    Wq = np.asarray(Wq, dtype=np.float32)
    Wk = np.asarray(Wk, dtype=np.float32)
    Wv = np.asarray(Wv, dtype=np.float32)
    Wo = np.asarray(Wo, dtype=np.float32)
    B, S, E = x.shape
    H, DH, _ = Wq.shape
    HPC = H // 2

    key = (S, E, HPC, DH)
    if key not in _NC_CACHE:
        _NC_CACHE[key] = build_nc(S=S, E=E, HPC=HPC, DH=DH)
    nc = _NC_CACHE[key]

    in_maps = _host_prep(x, Wq, Wk, Wv, Wo, HPC=HPC, DH=DH)
    res = run_bass_kernel_spmd(nc, in_maps, core_ids=list(range(N_CORES)))
    kernel.last_results = res

    out = np.empty((B, S, E), dtype=np.float32)
    for b in range(B):
        out[b] = (res.results[2 * b]["out"].astype(np.float32)
                  + res.results[2 * b + 1]["out"].astype(np.float32))
    return out
